# revision 12
# baseline (speedup 1.0000x reference)
"""Trainium2 Bass kernel for nn_CardModel_GRUCell (L=512, N=4096, I=19, H=30).

Strategy (data-parallel over batch, 8 cores, B=512 per core, 2 batch groups):
  - Hidden state reparameterized as Hs = (1+h)/2 so every activation is a
    Sigmoid (tanh(x) = 2*sigmoid(2x)-1) and gate sums come out of one PSUM
    accumulation chain:
      psum rows  0:30  = r_pre
      psum rows 32:62  = -z_pre        (sigmoid -> zc = 1-z directly)
      psum rows 64:94  = i_n + b_ihn   (+ r*hn' added by a 3rd matmul)
      psum rows 96:126 = h_n + b_hhn
  - All matmuls share tile_position (0,0) (mixed positions in one PSUM
    accumulation group fail at runtime on trn2):
      mm_x : lhsT = wx [19,128],  rhs = x_t^T        (start=True)
      mm_h : lhsT = wh [32,128],  rhs = [Hs; 0; 1]
      mm_t1: lhsT = i30x [30,128] (I30 in cols 64:94), rhs = t1 (accumulate)
  - Per step per group: 2 ACT sigmoids, 4 DVE tensor_tensor ops.
  - x is pre-staged host-side as [L/4, 19, 4*B] so each DMA is one
    contiguous block of 4 timesteps and the matmul rhs is a column slice.
"""

import os
import sys

import numpy as np

for _p in ("/opt/trn_rl_repo", os.path.expanduser("~/.axon_site/_ro/trn_rl_repo")):
    if os.path.isdir(_p) and _p not in sys.path:
        sys.path.insert(0, _p)

import concourse.bacc as bacc
import concourse.bass as bass
import concourse.tile as tile
from concourse import mybir
from concourse.bass_utils import run_bass_kernel_spmd

F32 = mybir.dt.float32
AF = mybir.ActivationFunctionType
ALU = mybir.AluOpType

NCORES = 8
GROUPS = 2
TBLK = 4  # timesteps per x DMA block


def build_program(L: int, B: int):
    assert L % TBLK == 0
    BG = B // GROUPS
    nc = bacc.Bacc(None, target_bir_lowering=False)

    xs_d = nc.declare_dram_parameter("xs", [L // TBLK, 19, TBLK * B], F32, isOutput=False)
    wf_d = nc.declare_dram_parameter("wf", [128, 128], F32, isOutput=False)
    wfc_d = nc.declare_dram_parameter("wfc", [32, 2], F32, isOutput=False)
    h0_d = nc.declare_dram_parameter("h0T", [32, B], F32, isOutput=False)
    hT_d = nc.declare_dram_parameter("hT", [30, B], F32, isOutput=True)
    yT_d = nc.declare_dram_parameter("yT", [2, B], F32, isOutput=True)

    with tile.TileContext(nc) as tc:
        with (
            tc.tile_pool(name="consts", bufs=1) as consts,
            tc.tile_pool(name="state", bufs=1) as state,
            tc.tile_pool(name="xs", bufs=3) as xsp,
            tc.tile_pool(name="work", bufs=2) as work,
            tc.tile_pool(name="ps", bufs=2, space="PSUM") as psp,
            tc.tile_pool(name="psy", bufs=1, space="PSUM") as psyp,
        ):
            wx_sb = consts.tile([19, 128], F32, tag="wx")
            nc.sync.dma_start(out=wx_sb, in_=wf_d[0:19, :])
            wh_sb = consts.tile([32, 128], F32, tag="wh")
            nc.sync.dma_start(out=wh_sb, in_=wf_d[64:96, :])
            i30x_sb = consts.tile([30, 128], F32, tag="i30x")
            nc.sync.dma_start(out=i30x_sb, in_=wf_d[96:126, :])
            wfc_sb = consts.tile([32, 2], F32, tag="wfc")
            nc.sync.dma_start(out=wfc_sb, in_=wfc_d[:, :])

            hB = []
            for g in range(GROUPS):
                t = state.tile([32, BG], F32, tag=f"hB{g}")
                # rows 0:30 <- Hs0, row 30 <- 0, row 31 <- 1 (host-packed)
                nc.sync.dma_start(out=t, in_=h0_d[:, g * BG : (g + 1) * BG])
                hB.append(t)

            xs_t = None
            for t in range(L):
                blk, a = divmod(t, TBLK)
                if a == 0:
                    xs_t = xsp.tile([19, TBLK * B], F32, tag="xs")
                    nc.sync.dma_start(out=xs_t, in_=xs_d[blk])
                for g in range(GROUPS):
                    cs = slice(g * BG, (g + 1) * BG)
                    xcs = slice(a * B + g * BG, a * B + (g + 1) * BG)
                    ps = psp.tile([128, BG], F32, tag=f"ps{g}")
                    nc.tensor.matmul(
                        ps[0:128, :], wx_sb[:, :], xs_t[:, xcs],
                        start=True, stop=False,
                    )
                    nc.tensor.matmul(
                        ps[0:128, :], wh_sb[:, :], hB[g][:, :],
                        start=False, stop=True,
                    )
                    rz = work.tile([62, BG], F32, tag=f"rz{g}")
                    nc.scalar.activation(
                        out=rz[0:62, :], in_=ps[0:62, :], func=AF.Sigmoid
                    )
                    t1 = work.tile([30, BG], F32, tag=f"t1{g}")
                    nc.vector.tensor_tensor(
                        t1[:, :], rz[0:30, :], ps[96:126, :], ALU.mult
                    )
                    nc.tensor.matmul(
                        ps[0:128, :], i30x_sb[:, :], t1[:, :],
                        start=False, stop=True, skip_group_check=True,
                    )
                    u = work.tile([30, BG], F32, tag=f"u{g}")
                    nc.scalar.activation(
                        out=u[:, :], in_=ps[64:94, :], func=AF.Sigmoid, scale=2.0
                    )
                    # v parked at base 32 so the w-op's SBUF inputs share a base
                    v = work.tile([62, BG], F32, tag=f"v{g}")
                    nc.vector.tensor_tensor(
                        v[32:62, :], hB[g][0:30, :], u[:, :], ALU.subtract
                    )
                    w = work.tile([30, BG], F32, tag=f"w{g}")
                    nc.vector.tensor_tensor(
                        w[:, :], rz[32:62, :], v[32:62, :], ALU.mult
                    )
                    nc.vector.tensor_tensor(
                        hB[g][0:30, :], hB[g][0:30, :], w[:, :], ALU.subtract
                    )

            for g in range(GROUPS):
                cs = slice(g * BG, (g + 1) * BG)
                hf = work.tile([30, BG], F32, tag=f"hf{g}")
                nc.vector.tensor_scalar(
                    out=hf[:, :], in0=hB[g][0:30, :],
                    scalar1=2.0, scalar2=-1.0, op0=ALU.mult, op1=ALU.add,
                )
                nc.sync.dma_start(out=hT_d[:, cs], in_=hf[:, :])
                psy = psyp.tile([32, BG], F32, tag=f"psy{g}")
                nc.tensor.matmul(
                    psy[0:2, :], wfc_sb[:, :], hB[g][:, :], start=True, stop=True
                )
                y = work.tile([2, BG], F32, tag=f"y{g}")
                nc.scalar.activation(out=y[:, :], in_=psy[0:2, :], func=AF.Sigmoid)
                nc.sync.dma_start(out=yT_d[:, cs], in_=y[:, :])

    nc.finalize()
    return nc


def make_host_operands(x, h0, W_ih, W_hh, b_ih, b_hh, W_fc, b_fc):
    """Fold weights/biases into the device operands (numpy, fp32)."""
    x = np.asarray(x, np.float32)
    h0 = np.asarray(h0, np.float32)
    W_ih = np.asarray(W_ih, np.float32)
    W_hh = np.asarray(W_hh, np.float32)
    b_ih = np.asarray(b_ih, np.float32)
    b_hh = np.asarray(b_hh, np.float32)
    W_fc = np.asarray(W_fc, np.float32)
    b_fc = np.asarray(b_fc, np.float32)

    L, N, I = x.shape
    H = W_hh.shape[1]
    assert I == 19 and H == 30

    # Staged x: xs[b, i, a*N + n] = x[TBLK*b + a, n, i]
    xs = np.ascontiguousarray(
        x.reshape(L // TBLK, TBLK, N, I).transpose(0, 3, 1, 2).reshape(
            L // TBLK, I, TBLK * N
        )
    )

    wf = np.zeros((128, 128), np.float32)
    WihT = W_ih.T  # [19, 90]
    wf[0:19, 0:30] = WihT[:, 0:30]
    wf[0:19, 32:62] = -WihT[:, 30:60]
    wf[0:19, 64:94] = WihT[:, 60:90]
    WhhT = W_hh.T  # [30, 90]
    wf[64:94, 0:30] = 2.0 * WhhT[:, 0:30]
    wf[64:94, 32:62] = -2.0 * WhhT[:, 30:60]
    wf[64:94, 96:126] = 2.0 * WhhT[:, 60:90]
    s = W_hh.sum(axis=1)  # [90]
    wf[95, 0:30] = b_ih[0:30] + b_hh[0:30] - s[0:30]
    wf[95, 32:62] = -(b_ih[30:60] + b_hh[30:60]) + s[30:60]
    wf[95, 64:94] = b_ih[60:90]
    wf[95, 96:126] = b_hh[60:90] - s[60:90]
    # I30 block for the t1-accumulate matmul: rows 96:126, identity into
    # columns 64:94 (loaded to SBUF base 0 as a [30,128] lhsT).
    wf[np.arange(96, 126), np.arange(64, 94)] = 1.0

    wfc = np.zeros((32, 2), np.float32)
    wfc[0:30, :] = 2.0 * W_fc.T
    wfc[31, :] = b_fc - W_fc.sum(axis=1)

    Hs0 = np.zeros((32, N), np.float32)  # rows 0:30 Hs, row 30 = 0, row 31 = 1
    Hs0[0:30, :] = (0.5 + 0.5 * h0).T
    Hs0[31, :] = 1.0
    return xs, wf, wfc, Hs0


def kernel(x, h0, W_ih, W_hh, b_ih, b_hh, W_fc, b_fc):
    x = np.asarray(x, np.float32)
    L, N, I = x.shape
    B = N // NCORES
    xs, wf, wfc, Hs0 = make_host_operands(
        x, h0, W_ih, W_hh, b_ih, b_hh, W_fc, b_fc
    )
    # per-core column slices of the staged x: columns a*N+n with n in core range
    xs_v = xs.reshape(L // TBLK, 19, TBLK, N)

    nc = build_program(L, B)
    in_maps = []
    for c in range(NCORES):
        ns = slice(c * B, (c + 1) * B)
        in_maps.append(
            {
                "xs": np.ascontiguousarray(xs_v[:, :, :, ns]).reshape(
                    L // TBLK, 19, TBLK * B
                ),
                "wf": wf,
                "wfc": wfc,
                "h0T": np.ascontiguousarray(Hs0[:, ns]),
            }
        )
    res = run_bass_kernel_spmd(nc, in_maps, list(range(NCORES)))
    y = np.empty((N, 2), np.float32)
    h = np.empty((N, 30), np.float32)
    for c in range(NCORES):
        ns = slice(c * B, (c + 1) * B)
        y[ns] = res.results[c]["yT"].T
        h[ns] = res.results[c]["hT"].T
    kernel.last_results = res
    return y, h
